# revision 22
# baseline (speedup 1.0000x reference)
"""Trainium2 Bass kernel for the VQ-codebook encoding module.

Math (per batch b, with x = X[b] reshaped (D, N)):
    resid_k[d,n] = x[d,n] - c[k,d]
    A = softmax_k(s[k,d] * resid^2)
    E[d,n]  = sum_k A*resid = x - (sum_k e_k*c_k)/(sum_k e_k),  e_k = exp(s*resid^2)
    EM[d]   = (1/K) sum_n E[d,n]
    gamma   = sigmoid(EM @ fc_w.T + fc_b)
    out     = relu(E * (1+gamma))

Implementation notes:
  - data-parallel over B: one batch image per NeuronCore (8 cores).
  - k's processed in pairs packed on partitions: [0:64]=d for k=2j, [64:128]=d for k=2j+1.
  - scale folded into the residual so the exp has a constant affine:
        T' = x*alpha - beta, alpha=sqrt(-s), beta=c*alpha  ->  e = exp(-T'^2)
    letting one ACT exp op cover a group of pairs (merged free dim).
  - per-pair T'^2 on DVE (tensor_scalar + square) for most pairs, fused ACT
    Square for a few (engine balance); Square/Exp share one ACT table set.
  - contraction over k on the PE in fp8 DoubleRow mode: two pairs (4 k's)
    per matmul; stationary [128,2,128] = stacked identity(x64) / diag(c*64),
    accumulating S1*64 (cols 0-63) and S2*64 (cols 64-127) into PSUM f32.
  - epilogue per half: R=1/(64*S1) (fast approx recip), Mneg=-(64*S2)*R with
    row-sum accumulated; E = x + Mneg. EM comes from host-precomputed sum(x)
    plus the Mneg row-sums, so gamma is ready before E of the last half;
    final relu(E*(1+gamma)) is one tensor_scalar per half feeding its DMA.
"""

import numpy as np
import ml_dtypes
from contextlib import ExitStack

import concourse.bacc as bacc
import concourse.tile as tile
from concourse import mybir
from concourse.bass_utils import run_bass_kernel_spmd

BF16 = ml_dtypes.bfloat16
FP8 = ml_dtypes.float8_e4m3

B, D, HH, WW, K = 8, 64, 56, 56, 32
N = HH * WW            # 3136
NPAIR = K // 2         # 16
NDUO = NPAIR // 2      # 8
NCORES = 8
HALVES = 2
# symmetric column split (asymmetric splits tested worse: a larger first half
# inflates the pipeline-fill head more than the smaller tail saves)
NHS = [1568, 1568]
EXP_GROUP = 4          # pairs per merged exp op
MM_CHUNK = 512         # psum bank
WSCALE = 64.0          # fp8 weight scale (cancels in S2/S1)

# pairs whose (x*alpha-beta)^2 runs fully on ScalarE (engine balance)
ACT_J = frozenset({2, 7, 10, 13})
# merged-exp group sizes per half (even sizes; small first group starts the
# ACT pipeline early, small last group in half 1 shortens the tail)
GROUPS = [[2, 6, 4, 4], [4, 4, 6, 2]]

_CACHE = {}


def _build_module():
    nc = bacc.Bacc("TRN2", target_bir_lowering=False, debug=False)
    f32 = mybir.dt.float32
    bf = mybir.dt.bfloat16
    fp8 = mybir.dt.float8e4
    Alu = mybir.AluOpType
    Act = mybir.ActivationFunctionType
    DR = mybir.MatmulPerfMode.DoubleRow

    X2 = nc.dram_tensor("X2", [128, N], bf, kind="ExternalInput")
    W8 = nc.dram_tensor("W8", [128, NDUO * 2 * 128], fp8, kind="ExternalInput")
    AL = nc.dram_tensor("AL", [128, NPAIR], f32, kind="ExternalInput")
    NBE = nc.dram_tensor("NBE", [128, NPAIR], f32, kind="ExternalInput")
    FW = nc.dram_tensor("FW", [64, 64], f32, kind="ExternalInput")
    NB = nc.dram_tensor("NB", [64, 1], f32, kind="ExternalInput")
    XS = nc.dram_tensor("XS", [64, 1], f32, kind="ExternalInput")
    Y = nc.dram_tensor("Y", [64, N], f32, kind="ExternalOutput")

    with tile.TileContext(nc) as tc, ExitStack() as ctx:
        const = ctx.enter_context(tc.tile_pool(name="const", bufs=1))
        x2p = ctx.enter_context(tc.tile_pool(name="x2p", bufs=2))
        tpp = ctx.enter_context(tc.tile_pool(name="tpp", bufs=4))
        qpp = ctx.enter_context(tc.tile_pool(name="qpp", bufs=3))
        epp = ctx.enter_context(tc.tile_pool(name="epp", bufs=3))
        wrk = ctx.enter_context(tc.tile_pool(name="wrk", bufs=2))
        ep2 = ctx.enter_context(tc.tile_pool(name="ep2", bufs=1))
        sml = ctx.enter_context(tc.tile_pool(name="sml", bufs=10))
        psum = ctx.enter_context(tc.tile_pool(name="psum", bufs=1, space="PSUM"))
        gps = ctx.enter_context(tc.tile_pool(name="gpsum", bufs=1, space="PSUM"))

        # warm the ACT exp table during the DMA head so the first real
        # ACTIVATE doesn't serialize behind the ~1.3us table load
        warm = sml.tile([64, 1], f32, tag="warm")
        nc.vector.memset(warm[:], 0.0)
        nc.scalar.activation(out=warm[:], in_=warm[:], func=Act.Exp, scale=-1.0)

        # DMA order: half-0 x + the per-pair scalars first so compute starts
        # as early as possible; everything else behind them.
        sx2s = []
        sAL = const.tile([128, NPAIR], f32)
        nc.sync.dma_start(out=sAL[:], in_=AL.ap())
        sNBE = const.tile([128, NPAIR], f32)
        nc.sync.dma_start(out=sNBE[:], in_=NBE.ap())
        sx2 = x2p.tile([128, NHS[0]], bf, tag="x2h0")
        nc.sync.dma_start(out=sx2[0:64, :], in_=X2.ap()[0:64, 0:NHS[0]])
        nc.sync.dma_start(out=sx2[64:128, :], in_=X2.ap()[64:128, 0:NHS[0]])
        sx2s.append(sx2)
        sx2 = x2p.tile([128, NHS[1]], bf, tag="x2h1")
        nc.sync.dma_start(out=sx2[0:64, :], in_=X2.ap()[0:64, NHS[0]:N])
        nc.sync.dma_start(out=sx2[64:128, :], in_=X2.ap()[64:128, NHS[0]:N])
        sx2s.append(sx2)
        sW8 = const.tile([128, NDUO, 2, 128], fp8)
        nc.sync.dma_start(out=sW8[:], in_=W8.ap().rearrange("p (g k m) -> p g k m",
                                                            g=NDUO, k=2))
        sFW = const.tile([64, 64], f32)
        nc.sync.dma_start(out=sFW[:], in_=FW.ap())
        sNB = const.tile([64, 1], f32)
        nc.sync.dma_start(out=sNB[:], in_=NB.ap())
        sXS = const.tile([64, 1], f32)
        nc.sync.dma_start(out=sXS[:], in_=XS.ap())

        e32s = []
        em_halves = []

        for h in range(HALVES):
            nh = NHS[h]
            n0 = sum(NHS[:h])
            sx2 = sx2s[h]
            ph = psum.tile([128, nh], f32, tag="mainpsum")

            j0 = 0
            gmax = max(max(gs) for gs in GROUPS)
            for gsz in GROUPS[h]:
                qtf = qpp.tile([128, gmax, nh], bf, tag="qt")
                qt = qtf[:, 0:gsz]
                for jj in range(gsz):
                    j = j0 + jj
                    al = sAL[:, j:j + 1]
                    nb = sNBE[:, j:j + 1]
                    if j in ACT_J:
                        nc.scalar.activation(out=qt[:, jj], in_=sx2[:], func=Act.Square,
                                             scale=al, bias=nb)
                    else:
                        tp = tpp.tile([128, nh], bf, tag="tprime")
                        nc.vector.tensor_scalar(out=tp[:], in0=sx2[:], scalar1=al,
                                                scalar2=nb, op0=Alu.mult, op1=Alu.add)
                        nc.vector.tensor_tensor(out=qt[:, jj], in0=tp[:], in1=tp[:],
                                                op=Alu.mult)
                last_group = (h == HALVES - 1) and (j0 + gsz == NPAIR)
                if last_group:
                    # split the tail-critical exp into column halves on separate
                    # tiles so the final matmuls + recip chunk 0 start early
                    hc = (nh // 2 + MM_CHUNK - 1) // MM_CHUNK * MM_CHUNK
                    ets = []
                    for (e0, e1) in ((0, hc), (hc, nh)):
                        etp = epp.tile([128, gmax, hc], fp8, tag=f"etl{e0}")
                        nc.scalar.activation(out=etp[:, 0:gsz, 0:e1 - e0],
                                             in_=qt[:, :, e0:e1], func=Act.Exp,
                                             scale=-1.0)
                        ets.append(etp)
                    for dd in range(gsz // 2):
                        duo = j0 // 2 + dd
                        for c0 in range(0, nh, MM_CHUNK):
                            c1 = min(c0 + MM_CHUNK, nh)
                            part = 0 if c1 <= hc else 1
                            b0 = c0 - part * hc
                            nc.tensor.matmul(ph[:, c0:c1], lhsT=sW8[:, duo, :, :],
                                             rhs=ets[part][:, 2 * dd:2 * dd + 2,
                                                           b0:b0 + (c1 - c0)],
                                             perf_mode=DR,
                                             start=(duo == 0), stop=(duo == NDUO - 1))
                else:
                    etf = epp.tile([128, gmax, nh], fp8, tag="et")
                    et = etf[:, 0:gsz]
                    nc.scalar.activation(out=et[:], in_=qt[:], func=Act.Exp, scale=-1.0)
                    for dd in range(gsz // 2):
                        duo = j0 // 2 + dd
                        for c0 in range(0, nh, MM_CHUNK):
                            c1 = min(c0 + MM_CHUNK, nh)
                            nc.tensor.matmul(ph[:, c0:c1], lhsT=sW8[:, duo, :, :],
                                             rhs=et[:, 2 * dd:2 * dd + 2, c0:c1],
                                             perf_mode=DR,
                                             start=(duo == 0), stop=(duo == NDUO - 1))
                j0 += gsz

            # epilogue for this half, in 2 column chunks: PSUM deps are
            # bank-level, so chunk 0's reciprocal starts before the last
            # matmuls of the upper banks complete, and the stt chain pipelines
            rt = wrk.tile([64, nh], f32, tag="recip")
            mn = wrk.tile([64, nh], f32, tag="prod")  # -(64*S2)*R
            e32 = ep2.tile([64, nh], f32, tag=f"e32h{h}")
            EC = nh // 2
            for q in range(2):
                c0, c1 = q * EC, (q + 1) * EC
                nc.vector.reciprocal_approx_fast(out=rt[:, c0:c1], in_=ph[0:64, c0:c1])
                emh = sml.tile([64, 1], f32, tag=f"em{h}q{q}")
                nc.vector.scalar_tensor_tensor(out=mn[:, c0:c1], in0=ph[64:128, c0:c1],
                                               scalar=-1.0, in1=rt[:, c0:c1],
                                               op0=Alu.mult, op1=Alu.mult,
                                               accum_out=emh[:])
                em_halves.append(emh)
                # E = x + Mneg
                nc.vector.scalar_tensor_tensor(out=e32[:, c0:c1], in0=mn[:, c0:c1],
                                               scalar=0.0, in1=sx2[0:64, c0:c1],
                                               op0=Alu.add, op1=Alu.add)
            e32s.append(e32)

        # gamma (depends only on XS and the Mneg row-sums)
        acc = sXS
        for i, emh in enumerate(em_halves):
            nxt = sml.tile([64, 1], f32, tag=f"emacc{i}")
            nc.vector.tensor_tensor(out=nxt[:], in0=acc[:], in1=emh[:], op=Alu.add)
            acc = nxt
        em = acc
        gp = gps.tile([64, 1], f32)
        nc.tensor.matmul(gp[:], lhsT=sFW[:], rhs=em[:], start=True, stop=True)
        ut = sml.tile([64, 1], f32, tag="ut")
        nc.scalar.activation(out=ut[:], in_=gp[:], func=Act.Exp, scale=-1.0, bias=sNB[:])
        vt = sml.tile([64, 1], f32, tag="vt")
        nc.vector.tensor_scalar_add(vt[:], ut[:], 1.0)
        wt = sml.tile([64, 1], f32, tag="wt")
        nc.vector.reciprocal(wt[:], vt[:])
        ft = sml.tile([64, 1], f32, tag="ft")
        nc.vector.tensor_scalar_add(ft[:], wt[:], 1.0)

        # final: relu(E*(1+gamma)) -> DMA, in quarter-chunks so the output
        # DMAs pipeline behind the scale op
        for h in range(HALVES):
            nh = NHS[h]
            n0 = sum(NHS[:h])
            nq = nh // 2
            yt = ep2.tile([64, nh], f32, tag=f"yth{h}")
            for q in range(2):
                c0 = q * nq
                nc.vector.tensor_scalar(out=yt[:, c0:c0 + nq],
                                        in0=e32s[h][:, c0:c0 + nq], scalar1=ft[:],
                                        scalar2=0.0, op0=Alu.mult, op1=Alu.max)
                nc.sync.dma_start(out=Y.ap()[:, n0 + c0:n0 + c0 + nq],
                                  in_=yt[:, c0:c0 + nq])

    nc.compile()
    return nc


def _host_prep(X, codewords, scale, fc_w, fc_b):
    Xr = X.reshape(B, D, N).astype(np.float32)
    alpha = np.sqrt(np.maximum(-scale.astype(np.float64), 0.0)).astype(np.float32)  # (K,D)
    nbeta = (-(codewords.astype(np.float64) * alpha.astype(np.float64))).astype(np.float32)

    AL = np.zeros((128, NPAIR), np.float32)
    NBE = np.zeros((128, NPAIR), np.float32)
    W8 = np.zeros((128, NDUO, 2, 128), np.float32)
    eye64 = np.eye(64, dtype=np.float32) * WSCALE
    for j in range(NPAIR):
        AL[0:64, j] = alpha[2 * j]
        AL[64:128, j] = alpha[2 * j + 1]
        NBE[0:64, j] = nbeta[2 * j]
        NBE[64:128, j] = nbeta[2 * j + 1]
        duo, ko = divmod(j, 2)
        W8[0:64, duo, ko, 0:64] = eye64
        W8[64:128, duo, ko, 0:64] = eye64
        W8[0:64, duo, ko, 64:128] = np.diag(codewords[2 * j]) * WSCALE
        W8[64:128, duo, ko, 64:128] = np.diag(codewords[2 * j + 1]) * WSCALE
    W8 = W8.reshape(128, NDUO * 2 * 128).astype(FP8)
    FW = (fc_w.T.astype(np.float32) / K).copy()
    NB = (-fc_b.astype(np.float32)).reshape(64, 1).copy()

    in_maps = []
    for b in range(B):
        Xb_bf = Xr[b].astype(BF16)
        X2 = np.concatenate([Xb_bf, Xb_bf], axis=0)
        # host-precomputed sum_n x (bf16-rounded x, matching the device E path)
        XSb = Xb_bf.astype(np.float32).sum(axis=1, keepdims=True)
        in_maps.append({
            "X2": X2,
            "W8": W8,
            "AL": AL,
            "NBE": NBE,
            "FW": FW,
            "NB": NB,
            "XS": XSb,
        })
    return in_maps


def kernel(X, codewords, scale, fc_w, fc_b):
    if "nc" not in _CACHE:
        _CACHE["nc"] = _build_module()
    nc = _CACHE["nc"]
    in_maps = _host_prep(np.asarray(X), np.asarray(codewords), np.asarray(scale),
                         np.asarray(fc_w), np.asarray(fc_b))
    res = run_bass_kernel_spmd(nc, in_maps, core_ids=list(range(NCORES)))
    out = np.stack([res.results[c]["Y"].reshape(D, HH, WW) for c in range(NCORES)])
    return out.astype(np.float32)
